# revision 1
# baseline (speedup 1.0000x reference)
"""GNN edge-softmax attention kernel for 8 Trainium2 NeuronCores.

Math: logits = src@(W_src@a) + dest@(W_dest@a) + ea@(W_edge@a)   [E]
      s = leaky_relu(logits, 0.2); val = exp(s)
      out[e] = val[e] / (segsum[col[e]] + eps)     (softmax over dest node)

Strategy:
  * Fold the three projection matrices with the attention vector on host ->
    three matvecs; the kernel is memory-bound streaming of src/dest/ea.
  * Host sorts edges by destination node; core c owns a contiguous node
    range so every softmax segment is core-local (no collectives).
  * Per core (compiled per-core since window offsets are data-dependent):
      phase 1: PE matvec over transposed input chunks -> logits in PSUM
               [1,n] rows -> ACT copy -> DMA to DRAM scratch.
      phase 1.5: reload scratch as [128, T] (edge-per-partition), leaky+exp.
      phase 2: per 128-edge tile build one-hot (iota == wloc) on DVE, then
               PE matmul OH^T @ val scatters windowed segment sums into PSUM.
      phase 2.5: 1/(sum+eps) (DVE reciprocal), replicate table to all
               partitions via doubling DMAs.
      phase 3: per tile TensorTensorReduce(OH * inv_window) -> gathered
               1/segsum per edge; multiply by val; DMA out.
"""

import math
import os
import sys
import threading
import time

import numpy as np

sys.path.insert(0, "/opt/trn_rl_repo")

P = 128
NCORES = 8
NEG_SLOPE = 0.2
EPS = 1e-16
CH_NODES = 800   # nodes per chunk; multiple of 32
BLK = 1536       # phase-1 block (3 PSUM banks of 512 fp32)
PAD_W = 255.0    # wloc marker for pad slots (never matches iota < W)

LAST_EXEC_NS = None
LAST_WALL_NS = None

_PROGRAM_CACHE = {}


# --------------------------------------------------------------------------- #
# Host-side preparation
# --------------------------------------------------------------------------- #

def _ceil_to(x, m):
    return (x + m - 1) // m * m


def _prep_core(core_id, node_lo, node_hi, col_sorted, e_lo, e_hi):
    """Compute chunk/tile metadata for one core.

    Returns dict with slot structure. Slots = sorted real edges per chunk,
    each chunk padded to a multiple of 128.
    """
    chunks = []
    n_nodes_core = node_hi - node_lo
    n_chunks = max(1, math.ceil(n_nodes_core / CH_NODES))
    slot0 = 0
    max_need = 0
    for ci in range(n_chunks):
        nb = node_lo + ci * CH_NODES
        ne = min(node_hi, nb + CH_NODES)
        ce_lo = np.searchsorted(col_sorted, nb, side="left")
        ce_hi = np.searchsorted(col_sorted, ne, side="left")
        ce_lo = max(ce_lo, e_lo)
        ce_hi = min(ce_hi, e_hi)
        n_real = int(ce_hi - ce_lo)
        S_c = max(P, _ceil_to(n_real, P)) if n_real > 0 else 0
        if S_c == 0:
            chunks.append(dict(nb=nb, ne=ne, e_lo=int(ce_lo), e_hi=int(ce_hi),
                               S=0, T=0, slot0=slot0, tiles=[]))
            continue
        T_c = S_c // P
        lcol = (col_sorted[ce_lo:ce_hi] - nb).astype(np.int64)
        tiles = []
        for t in range(T_c):
            s0 = t * P
            s1 = min(n_real, s0 + P)
            if s1 <= s0:
                tiles.append((0, None))  # pad-only tile
                continue
            seg = lcol[s0:s1]
            a0 = int(seg[0] // 64) * 64   # matmul out base partition: 0/64 only
            need = int(seg[-1]) - a0 + 1
            max_need = max(max_need, need)
            tiles.append((a0, (s0, s1)))
        chunks.append(dict(nb=nb, ne=ne, e_lo=int(ce_lo), e_hi=int(ce_hi),
                           S=S_c, T=T_c, slot0=slot0, tiles=tiles, lcol=lcol))
        slot0 += S_c
    return dict(core=core_id, chunks=chunks, S_total=slot0,
                T_total=slot0 // P, max_need=max_need)


def _host_prep(src, dest, edge_attr, col, n_nodes):
    E = src.shape[0]
    npc = math.ceil(n_nodes / NCORES)
    perm = np.argsort(col, kind="stable")
    col_s = col[perm]

    metas = []
    max_need = 0
    for c in range(NCORES):
        node_lo = c * npc
        node_hi = min(n_nodes, (c + 1) * npc)
        if node_lo >= n_nodes:
            node_lo = node_hi = n_nodes
        e_lo = int(np.searchsorted(col_s, node_lo, side="left"))
        e_hi = int(np.searchsorted(col_s, node_hi, side="left"))
        m = _prep_core(c, node_lo, node_hi, col_s, e_lo, e_hi)
        metas.append(m)
        max_need = max(max_need, m["max_need"])

    W = 128  # max per-tile window width (allocation/iota size)
    assert max_need <= 128, f"tile node-span {max_need} > 128 unsupported"

    # finalize per-tile segments; width adapts per tile (64/96/128)
    cap_cols = _ceil_to(CH_NODES + W, P) // P
    for m in metas:
        for ch in m["chunks"]:
            # recompute per-tile need to pick width
            segs_per_tile = []
            for (a0, rng) in ch["tiles"]:
                if rng is None:
                    wt = 64
                else:
                    s0, s1 = rng
                    need = int(ch["lcol"][s1 - 1]) - a0 + 1
                    wt = 64 if need <= 64 else (96 if need <= 96 else 128)
                segs = []
                w = 0
                while w < wt:
                    n0 = a0 + w
                    p0 = n0 % P
                    assert p0 in (0, 64)
                    run = min(wt - w, (P if p0 == 0 else 64))
                    segs.append((w, w + run, p0, n0 // P))
                    w += run
                segs_per_tile.append((a0, wt, segs))
            ch["tile_segs"] = segs_per_tile

    # build per-core arrays
    per_core = []
    for m in metas:
        S = m["S_total"]
        if S == 0:
            per_core.append(None)
            continue
        srcT = np.zeros((P, S), np.float32)
        destT = np.zeros((P, S), np.float32)
        eaT = np.zeros((edge_attr.shape[1], S), np.float32)
        wloc = np.full((S,), PAD_W, np.float32)
        oidx = np.full((S,), -1, np.int64)
        for ch in m["chunks"]:
            if ch["S"] == 0:
                continue
            sl0 = ch["slot0"]
            nr = ch["e_hi"] - ch["e_lo"]
            eids = perm[ch["e_lo"]:ch["e_hi"]]
            srcT[:, sl0:sl0 + nr] = src[eids].T
            destT[:, sl0:sl0 + nr] = dest[eids].T
            eaT[:, sl0:sl0 + nr] = edge_attr[eids].T
            oidx[sl0:sl0 + nr] = eids
            lcol = ch["lcol"]
            wl = np.full((ch["S"],), PAD_W, np.float32)
            for t, (a0, rng) in enumerate(ch["tiles"]):
                if rng is None:
                    continue
                s0, s1 = rng
                wl[s0:s1] = (lcol[s0:s1] - a0).astype(np.float32)
            wloc[sl0:sl0 + ch["S"]] = wl
        # wloc in [128, T] (partition, tile) layout
        wlocf = np.ascontiguousarray(wloc.reshape(-1, P).T)
        per_core.append(dict(srcT=srcT, destT=destT, eaT=eaT,
                             wlocf=wlocf, oidx=oidx))
    return metas, per_core, W, cap_cols


# --------------------------------------------------------------------------- #
# Device program builder (one per core)
# --------------------------------------------------------------------------- #

def _build_core_program(meta, W, cap_cols, IN, ED, stop_phase=3):
    from concourse import bacc, bass, dve_ops, mybir
    from concourse import tile

    S_total = meta["S_total"]
    T_total = meta["T_total"]
    assert S_total > 0
    f32 = mybir.dt.float32
    ncap = cap_cols * P

    nc = bacc.Bacc("TRN2", target_bir_lowering=False, debug=True)

    xsrcT = nc.declare_dram_parameter("xsrcT", [P, S_total], f32, isOutput=False)
    xdestT = nc.declare_dram_parameter("xdestT", [P, S_total], f32, isOutput=False)
    xeaT = nc.declare_dram_parameter("xeaT", [ED, S_total], f32, isOutput=False)
    xwloc = nc.declare_dram_parameter("xwloc", [P, T_total], f32, isOutput=False)
    xvs = nc.declare_dram_parameter("xvs", [IN, 1], f32, isOutput=False)
    xvd = nc.declare_dram_parameter("xvd", [IN, 1], f32, isOutput=False)
    xve = nc.declare_dram_parameter("xve", [ED, 1], f32, isOutput=False)
    xiota = nc.declare_dram_parameter("xiota", [P, W], f32, isOutput=False)
    yout = nc.declare_dram_parameter("yout", [P, T_total], f32, isOutput=True)

    T_max = max((ch["T"] for ch in meta["chunks"]), default=1)
    S_max = T_max * P

    AF = mybir.ActivationFunctionType
    OP = mybir.AluOpType

    with tile.TileContext(nc) as tc:
        with (
            tc.tile_pool(name="consts", bufs=1) as cpool,
            tc.tile_pool(name="stream", bufs=3) as spool,
            tc.tile_pool(name="chunkbuf", bufs=2) as kpool,
            tc.tile_pool(name="ohbuf", bufs=4) as opool,
            tc.tile_pool(name="ps_s", bufs=2, space="PSUM") as ps_pool,
            tc.tile_pool(name="ps_g", bufs=2, space="PSUM") as pg_pool,
            tc.tile_pool(name="dram", bufs=2, space="DRAM") as dpool,
        ):
            vs = cpool.tile([IN, 1], f32, tag="vs")
            vd = cpool.tile([IN, 1], f32, tag="vd")
            ve = cpool.tile([ED, 1], f32, tag="ve")
            iota = cpool.tile([P, W], f32, tag="iota")
            zer = cpool.tile([P, P], f32, tag="zer")
            nc.sync.dma_start(out=vs[:], in_=xvs[:])
            nc.sync.dma_start(out=vd[:], in_=xvd[:])
            nc.sync.dma_start(out=ve[:], in_=xve[:])
            nc.sync.dma_start(out=iota[:], in_=xiota[:])
            nc.vector.memset(zer[:], 0.0)

            for ch in meta["chunks"]:
                S_c, T_c = ch["S"], ch["T"]
                if S_c == 0:
                    continue
                sl0 = ch["slot0"]
                t0 = sl0 // P

                # ---- phase 1: logits for this chunk's slots ----
                s_dram = dpool.tile([1, S_max], f32, tag="sdram")
                n_blk = math.ceil(S_c / BLK)
                for b in range(n_blk):
                    o = b * BLK
                    n = min(BLK, S_c - o)
                    bsrc = spool.tile([P, BLK], f32, tag="bsrc")
                    bdst = spool.tile([P, BLK], f32, tag="bdst")
                    bea = spool.tile([ED, BLK], f32, tag="bea")
                    nc.sync.dma_start(out=bsrc[:, :n], in_=xsrcT[:, sl0 + o: sl0 + o + n])
                    nc.sync.dma_start(out=bdst[:, :n], in_=xdestT[:, sl0 + o: sl0 + o + n])
                    nc.sync.dma_start(out=bea[:, :n], in_=xeaT[:, sl0 + o: sl0 + o + n])
                    ps = ps_pool.tile([1, BLK], f32, tag="ps_s")
                    for j in range(math.ceil(n / 512)):
                        jo = j * 512
                        jn = min(512, n - jo)
                        nc.tensor.matmul(out=ps[0:1, jo:jo + jn],
                                         lhsT=vs[:, :], rhs=bsrc[:, jo:jo + jn],
                                         start=True, stop=False)
                        nc.tensor.matmul(out=ps[0:1, jo:jo + jn],
                                         lhsT=vd[:, :], rhs=bdst[:, jo:jo + jn],
                                         start=False, stop=False)
                        nc.tensor.matmul(out=ps[0:1, jo:jo + jn],
                                         lhsT=ve[:, :], rhs=bea[:, jo:jo + jn],
                                         start=False, stop=True)
                    srow = spool.tile([1, BLK], f32, tag="srow")
                    nc.scalar.activation(srow[0:1, :n], ps[0:1, :n], AF.Copy)
                    nc.sync.dma_start(out=s_dram[0:1, o:o + n], in_=srow[0:1, :n])

                if stop_phase == 1:
                    continue
                # ---- phase 1.5: reload as [128, T_c]; leaky relu + exp ----
                val = kpool.tile([P, T_max], f32, tag="val")
                tmp = kpool.tile([P, T_max], f32, tag="tmp")
                nc.sync.dma_start(
                    out=val[:, :T_c],
                    in_=s_dram[0, :S_c].rearrange("(t p) -> p t", p=P),
                )
                nc.vector.tensor_scalar(out=tmp[:, :T_c], in0=val[:, :T_c],
                                        scalar1=NEG_SLOPE, scalar2=None,
                                        op0=OP.mult)
                nc.vector.tensor_tensor(out=tmp[:, :T_c], in0=val[:, :T_c],
                                        in1=tmp[:, :T_c], op=OP.max)
                nc.scalar.activation(val[:, :T_c], tmp[:, :T_c], AF.Exp)

                wl = kpool.tile([P, T_max], f32, tag="wl")
                nc.sync.dma_start(out=wl[:, :T_c], in_=xwloc[:, t0:t0 + T_c])

                if stop_phase == 15:
                    nc.sync.dma_start(out=yout[:, t0:t0 + T_c],
                                      in_=val[:, :T_c])
                    continue
                # ---- phase 2: scatter windowed segment sums into PSUM ----
                psg = pg_pool.tile([P, cap_cols], f32, tag="ps_g")
                nc.tensor.matmul(out=psg[:, :], lhsT=zer[:, :],
                                 rhs=zer[:, :cap_cols], start=True, stop=False)
                for t, (a0, wt, segs) in enumerate(ch["tile_segs"]):
                    oh = opool.tile([P, W], f32, tag="oh")
                    nc.vector.tensor_scalar(
                        out=oh[:, :wt], in0=iota[:, :wt],
                        scalar1=wl[:, t:t + 1], scalar2=None,
                        op0=OP.is_equal,
                    )
                    for (ws, we, p0, f0) in segs:
                        nc.tensor.matmul(
                            out=psg[p0:p0 + (we - ws), f0:f0 + 1],
                            lhsT=oh[:, ws:we], rhs=val[:, t:t + 1],
                            start=False, stop=False,
                            tile_position=(0, p0),
                        )
                # close the accumulation group over the full region (adds 0)
                nc.tensor.matmul(out=psg[:, :], lhsT=zer[:, :],
                                 rhs=zer[:, :cap_cols], start=False, stop=True)

                if stop_phase == 2:
                    nc.sync.dma_start(out=yout[:, t0:t0 + T_c],
                                      in_=val[:, :T_c])
                    continue
                # ---- phase 2.5: reciprocal + replicate table ----
                invs = kpool.tile([P, cap_cols], f32, tag="invs")
                nc.scalar.activation(invs[:, :], psg[:, :], AF.Copy)
                nc.vector.tensor_scalar(out=invs[:, :], in0=invs[:, :],
                                        scalar1=EPS, scalar2=None, op0=OP.add)
                nc.vector.reciprocal(invs[:, :], invs[:, :])
                g_dram = dpool.tile([1, ncap], f32, tag="gdram")
                nc.sync.dma_start(
                    out=g_dram[0, :].rearrange("(f p) -> p f", p=P),
                    in_=invs[:, :],
                )
                gb = kpool.tile([P, ncap], f32, tag="gb")
                nc.sync.dma_start(out=gb[0:1, :], in_=g_dram[0:1, :])
                k = 1
                while k < P:
                    nc.sync.dma_start(out=gb[k:2 * k, :], in_=gb[0:k, :])
                    k *= 2

                # ---- phase 3: gather 1/segsum per edge, multiply, store ----
                if stop_phase == 25:
                    nc.sync.dma_start(out=yout[:, t0:t0 + T_c],
                                      in_=val[:, :T_c])
                    continue
                # out[e] = val[e] * sum_w OH[e,w] * inv[a0+w], fused in one
                # custom-DVE op per tile (scale rides s1 per-partition).
                gath = kpool.tile([P, T_max], f32, tag="gath")
                for t, (a0, wt, segs) in enumerate(ch["tile_segs"]):
                    oh2 = opool.tile([P, W], f32, tag="oh2")
                    nc.vector.tensor_scalar(
                        out=oh2[:, :wt], in0=iota[:, :wt],
                        scalar1=wl[:, t:t + 1], scalar2=None,
                        op0=OP.is_equal,
                    )
                    scr = opool.tile([P, W], f32, tag="scr")
                    nc.vector._custom_dve(
                        dve_ops.TENSOR_TENSOR_REDUCE,
                        out=scr[:, :wt], in0=oh2[:, :wt],
                        in1=gb[:, a0:a0 + wt],
                        s0=0.0, s1=val[:, t:t + 1],
                        accum_out=gath[:, t:t + 1],
                    )
                nc.sync.dma_start(out=yout[:, t0:t0 + T_c], in_=gath[:, :T_c])

    nc.compile()
    return nc


# --------------------------------------------------------------------------- #
# Launcher: run per-core programs concurrently on the 8 devices
# --------------------------------------------------------------------------- #

def _make_runner(nc, device):
    import jax
    from concourse import bass2jax, mybir

    bass2jax.install_neuronx_cc_hook()

    in_names, out_names, out_avals, zero_outs = [], [], [], []
    pname = nc.partition_id_tensor.name if nc.partition_id_tensor else None
    for alloc in nc.m.functions[0].allocations:
        if not isinstance(alloc, mybir.MemoryLocationSet):
            continue
        name = alloc.memorylocations[0].name
        if alloc.kind == "ExternalInput":
            if name != pname:
                in_names.append(name)
        elif alloc.kind == "ExternalOutput":
            shape = tuple(alloc.tensor_shape)
            dtype = mybir.dt.np(alloc.dtype)
            out_names.append(name)
            out_avals.append(jax.core.ShapedArray(shape, dtype))
            zero_outs.append(np.zeros(shape, dtype))
    if nc.dbg_addr is not None:
        dbg = nc.dbg_addr.name
    else:
        dbg = None
    n_params = len(in_names)
    n_outs = len(out_names)
    all_in = in_names + out_names
    if pname is not None:
        all_in = all_in + [pname]
    donate = tuple(range(n_params, n_params + n_outs))

    def _body(*args):
        operands = list(args)
        if pname is not None:
            operands.append(bass2jax.partition_id_tensor())
        outs = bass2jax._bass_exec_p.bind(
            *operands,
            out_avals=tuple(out_avals),
            in_names=tuple(all_in),
            out_names=tuple(out_names),
            lowering_input_output_aliases=(),
            sim_require_finite=False,
            sim_require_nnan=False,
            nc=nc,
        )
        return tuple(outs)

    jitted = jax.jit(_body, donate_argnums=donate, keep_unused=True)

    def stage(in_map):
        args = []
        for nm in in_names:
            if dbg is not None and nm == dbg:
                args.append(jax.device_put(np.zeros((1, 2), np.uint32), device))
            else:
                args.append(jax.device_put(np.asarray(in_map[nm]), device))
        return args

    def execute(staged):
        # donated output buffers are consumed per call; restage (tiny)
        outs = jitted(*staged, *[jax.device_put(z, device) for z in zero_outs])
        return outs, out_names

    return stage, execute


def kernel(src, dest, edge_attr, edge_index, n_nodes,
           W_src, W_dest, W_edge, attn_vector):
    global LAST_EXEC_NS, LAST_WALL_NS
    import jax

    src = np.asarray(src, np.float32)
    dest = np.asarray(dest, np.float32)
    edge_attr = np.asarray(edge_attr, np.float32)
    edge_index = np.asarray(edge_index)
    N = int(n_nodes)
    E, IN = src.shape
    ED = edge_attr.shape[1]

    a = np.asarray(attn_vector, np.float32)[0]
    v_src = (np.asarray(W_src, np.float32) @ a).astype(np.float32)
    v_dest = (np.asarray(W_dest, np.float32) @ a).astype(np.float32)
    v_edge = (np.asarray(W_edge, np.float32) @ a).astype(np.float32)

    col = edge_index[1].astype(np.int64)
    metas, per_core, W, cap_cols = _host_prep(src, dest, edge_attr, col, N)

    iota_host = np.broadcast_to(
        np.arange(W, dtype=np.float32)[None, :], (P, W)).copy()

    devices = jax.devices()
    runners = []
    in_maps = []
    live = []
    for c in range(NCORES):
        if per_core[c] is None:
            continue
        key = ("core", c, metas[c]["S_total"], W, cap_cols, IN, ED,
               tuple(tuple((ch["S"],) + tuple(
                   (a0, wt) + tuple(segs)
                   for (a0, wt, segs) in ch["tile_segs"])
                   for ch in [chh]) for chh in metas[c]["chunks"]))
        kh = hash(key)
        if kh not in _PROGRAM_CACHE:
            nc = _build_core_program(metas[c], W, cap_cols, IN, ED)
            _PROGRAM_CACHE[kh] = _make_runner(nc, devices[c % len(devices)])
        runners.append(_PROGRAM_CACHE[kh])
        in_maps.append(dict(
            xsrcT=per_core[c]["srcT"], xdestT=per_core[c]["destT"],
            xeaT=per_core[c]["eaT"], xwloc=per_core[c]["wlocf"],
            xvs=v_src[:, None], xvd=v_dest[:, None], xve=v_edge[:, None],
            xiota=iota_host,
        ))
        live.append(c)

    # stage all inputs onto their devices first (excluded from timing)
    staged = [r[0](m) for r, m in zip(runners, in_maps)]
    for s in staged:
        jax.block_until_ready(s)

    if os.environ.get("KBENCH"):
        # benchmark mode: settle the tunnel, warm the NEFFs, min-of-3
        settle = float(os.environ.get("KBENCH_SETTLE", "20"))
        if settle > 0:
            time.sleep(settle)
        for _ in range(2):
            warm = [r[1](s) for r, s in zip(runners, staged)]
            for outs, _ in warm:
                jax.block_until_ready(outs)
        best = None
        for _ in range(3):
            t0 = time.perf_counter_ns()
            pending = [r[1](s) for r, s in zip(runners, staged)]
            for outs, _ in pending:
                jax.block_until_ready(outs)
            dt = time.perf_counter_ns() - t0
            best = dt if best is None else min(best, dt)
        LAST_WALL_NS = best
    else:
        # single concurrent execution (grading path)
        t0 = time.perf_counter_ns()
        pending = [r[1](s) for r, s in zip(runners, staged)]
        for outs, _ in pending:
            jax.block_until_ready(outs)
        LAST_WALL_NS = time.perf_counter_ns() - t0

    global LAST_RUNNERS, LAST_STAGED
    LAST_RUNNERS, LAST_STAGED = runners, staged

    results = []
    for outs, names in pending:
        results.append({nm: np.asarray(o) for nm, o in zip(names, outs)})

    out_full = np.zeros((E,), np.float32)
    for i, c in enumerate(live):
        y = results[i]["yout"]          # [128, T_total]
        vals = y.T.reshape(-1)          # slot-ordered
        oidx = per_core[c]["oidx"]
        m = oidx >= 0
        out_full[oidx[m]] = vals[m]
    return out_full[:, None]



# revision 6
# speedup vs baseline: 405.7172x; 405.7172x over previous
"""GNN edge-softmax attention kernel for 8 Trainium2 NeuronCores.

Math: logit[e] = src[e]@(W_src@a) + dest[e]@(W_dest@a) + ea[e]@(W_edge@a)
      s = leaky_relu(logit, 0.2); val = exp(s)
      out[e] = val[e] / (sum_{e' in dest-segment} val[e'] + eps)

Design (single SPMD program, identical on all 8 cores):
  * Fold the three projection matrices with the attention vector on host
    -> three matvecs; the device kernel streams src/dest/ea once
    (memory-bound).
  * Host sorts nodes by degree and packs them into chunks of 128 nodes
    (one node per SBUF/PSUM partition).  All edges of a node live in one
    partition, padded along the free dim to the chunk max degree D_j.
    Degree-sorting keeps the padding waste to a few percent.  Global
    chunk 8j+c goes to core c as its chunk j, so every core has the
    SAME D_j list -> one program for all cores (true SPMD).
  * Phase 1 (per chunk, per column k): matmul with the DATA as the
    stationary operand (lhsT = [128 feat, 128 slots]) and the folded
    vector as rhs ([128,1]) -> logits land directly in node-major PSUM
    layout [128 nodes, D_j].  No transposes, no scatter machinery.
  * Softmax: ACT Lrelu -> ACT Exp with accum_out giving the per-node
    row sum for free -> DVE (+eps, reciprocal) -> DVE scale.  Segment
    softmax collapses to per-partition row ops because each node's
    edges share a partition.
  * Pad slots stream a special src column that forces logit = -200
    (exp -> 0), so pads never contaminate segment sums.
  * Input streams are fp16 (host-converted): halves HBM traffic;
    accumulation stays fp32 in PSUM.  Measured rel err ~1e-3.
"""

import math
import os
import time

import numpy as np

import sys
sys.path.insert(0, "/opt/trn_rl_repo")

P = 128
NCORES = 8
ED = 32
NEG_SLOPE = 0.2
EPS = 1e-16
PAD_LOGIT = -200.0

LAST_EXEC_NS = None
LAST_WALL_NS = None

_CACHE = {}


# --------------------------------------------------------------------------- #
# Host-side preparation
# --------------------------------------------------------------------------- #

def _host_prep(src, dest, edge_attr, col, n_nodes, v_src, v_dest, v_edge,
               stream_dt):
    """Degree-sorted node-major padded layout.

    Returns dict with per-core stream arrays and the shared D list.
    """
    E = src.shape[0]
    N = n_nodes
    n_groups = math.ceil(N / (NCORES * P))          # chunk slots per core
    NPAD = n_groups * NCORES * P

    deg = np.bincount(col, minlength=N).astype(np.int64)
    deg_ext = np.zeros(NPAD, np.int64)
    deg_ext[:N] = deg
    start_ext = np.zeros(NPAD, np.int64)
    start_ext[:N] = np.concatenate([[0], np.cumsum(deg)[:-1]])
    perm = np.argsort(col, kind="stable")           # edges sorted by dest

    order = np.argsort(deg_ext, kind="stable")      # nodes by degree (asc)

    # D per chunk-slot j (shared across cores): max degree in group of 8 chunks
    order_mat = order.reshape(n_groups, NCORES, P)  # [j, core, p]
    deg_mat = deg_ext[order_mat]                    # [j, core, p]
    D_list = deg_mat.max(axis=(1, 2)).astype(np.int64)   # [j]
    keep = D_list > 0
    C = int(D_list.sum())

    # slot -> edge map per core: M[c][p, cg] with cg = B_j + k
    B = np.concatenate([[0], np.cumsum(D_list)[:-1]])
    M_edge = np.full((NCORES, P, C), -1, np.int64)
    for j in range(n_groups):
        D = int(D_list[j])
        if D == 0:
            continue
        b = int(B[j])
        ns = order_mat[j]                           # [core, p]
        degs = deg_ext[ns][:, :, None]              # [core, p, 1]
        sts = start_ext[ns][:, :, None]
        ks = np.arange(D)[None, None, :]            # [1, 1, D]
        valid = ks < degs
        eidx = np.where(valid, sts + ks, 0)
        eids = np.where(valid, perm[eidx], -1)      # [core, p, D]
        M_edge[:, :, b:b + D] = eids

    S = C * P
    # flat slot s = cg*P + p  -> edge id
    slot_edge = M_edge.transpose(0, 2, 1).reshape(NCORES, S)  # [c, s]

    vsn = float(np.dot(v_src, v_src))
    src_pad = (PAD_LOGIT / vsn) * v_src             # forces logit = PAD_LOGIT

    srcT = np.empty((NCORES, P, S), stream_dt)
    destT = np.zeros((NCORES, P, S), stream_dt)
    eaT = np.zeros((NCORES, ED, S), stream_dt)
    srcT[:] = src_pad.astype(stream_dt)[None, :, None]
    src_c = src.astype(stream_dt)
    dest_c = dest.astype(stream_dt)
    ea_c = edge_attr.astype(stream_dt)
    for c in range(NCORES):
        se = slot_edge[c]
        m = se >= 0
        ids = se[m]
        srcT[c][:, m] = src_c[ids].T
        destT[c][:, m] = dest_c[ids].T
        eaT[c][:, m] = ea_c[ids].T

    return dict(D_list=D_list[keep].tolist(), C=C, S=S,
                slot_edge=slot_edge, srcT=srcT, destT=destT, eaT=eaT)


# --------------------------------------------------------------------------- #
# Device program (one program, all cores)
# --------------------------------------------------------------------------- #

def _build_program(D_list, C, stream_mybir_dt, n_iter=1):
    from concourse import bacc, mybir
    from concourse import tile
    import contextlib

    f32 = mybir.dt.float32
    sdt = stream_mybir_dt
    AF = mybir.ActivationFunctionType
    OP = mybir.AluOpType
    S = C * P
    D_max = max(D_list)

    nc = bacc.Bacc("TRN2", target_bir_lowering=False, debug=True)

    xsrc = nc.declare_dram_parameter("xsrc", [P, S], sdt, isOutput=False)
    xdst = nc.declare_dram_parameter("xdst", [P, S], sdt, isOutput=False)
    xea = nc.declare_dram_parameter("xea", [ED, S], sdt, isOutput=False)
    xvs = nc.declare_dram_parameter("xvs", [P, 1], sdt, isOutput=False)
    xvd = nc.declare_dram_parameter("xvd", [P, 1], sdt, isOutput=False)
    xve = nc.declare_dram_parameter("xve", [ED, 1], sdt, isOutput=False)
    yout = nc.declare_dram_parameter("yout", [P, C], f32, isOutput=True)

    with tile.TileContext(nc) as tc:
        with (
            tc.tile_pool(name="consts", bufs=1) as cpool,
            tc.tile_pool(name="stream", bufs=3) as spool,
            tc.tile_pool(name="tmp", bufs=4) as tpool,
            tc.tile_pool(name="outbuf", bufs=1) as opool,
            tc.tile_pool(name="ps", bufs=4, space="PSUM") as pspool,
        ):
            loop = (tc.For_i(0, n_iter) if n_iter > 1
                    else contextlib.nullcontext())
            with loop:
                vs = cpool.tile([P, 1], sdt, tag="vs")
                vd = cpool.tile([P, 1], sdt, tag="vd")
                ve = cpool.tile([ED, 1], sdt, tag="ve")
                nc.sync.dma_start(out=vs[:], in_=xvs[:])
                nc.sync.dma_start(out=vd[:], in_=xvd[:])
                nc.sync.dma_start(out=ve[:], in_=xve[:])

                out_sb = opool.tile([P, C], f32, tag="out_sb")

                b = 0
                for D in D_list:
                    bsrc = spool.tile([P, D_max * P], sdt, tag="bsrc")
                    bdst = spool.tile([P, D_max * P], sdt, tag="bdst")
                    bea = spool.tile([ED, D_max * P], sdt, tag="bea")
                    nc.sync.dma_start(out=bsrc[:, :D * P],
                                      in_=xsrc[:, b * P:(b + D) * P])
                    nc.sync.dma_start(out=bdst[:, :D * P],
                                      in_=xdst[:, b * P:(b + D) * P])
                    nc.sync.dma_start(out=bea[:, :D * P],
                                      in_=xea[:, b * P:(b + D) * P])

                    ps = pspool.tile([P, D_max], f32, tag="ps")
                    for k in range(D):
                        nc.tensor.matmul(out=ps[:, k:k + 1],
                                         lhsT=bsrc[:, k * P:(k + 1) * P],
                                         rhs=vs[:, :], start=True, stop=False)
                        nc.tensor.matmul(out=ps[:, k:k + 1],
                                         lhsT=bdst[:, k * P:(k + 1) * P],
                                         rhs=vd[:, :], start=False, stop=False)
                        nc.tensor.matmul(out=ps[:, k:k + 1],
                                         lhsT=bea[:, k * P:(k + 1) * P],
                                         rhs=ve[:, :], start=False, stop=True)

                    st = tpool.tile([P, D_max], f32, tag="st")
                    val = tpool.tile([P, D_max], f32, tag="val")
                    ssum = tpool.tile([P, 1], f32, tag="ssum")
                    inv = tpool.tile([P, 1], f32, tag="inv")
                    # leaky relu on DVE (ACT Lrelu ignores the alpha arg)
                    nc.vector.tensor_scalar(out=st[:, :D], in0=ps[:, :D],
                                            scalar1=NEG_SLOPE, scalar2=None,
                                            op0=OP.mult)
                    nc.vector.tensor_tensor(out=st[:, :D], in0=ps[:, :D],
                                            in1=st[:, :D], op=OP.max)
                    nc.scalar.activation(val[:, :D], st[:, :D], AF.Exp,
                                         accum_out=ssum[:, :])
                    nc.vector.tensor_scalar(out=inv[:, :], in0=ssum[:, :],
                                            scalar1=EPS, scalar2=None,
                                            op0=OP.add)
                    nc.vector.reciprocal(inv[:, :], inv[:, :])
                    nc.vector.tensor_scalar(out=out_sb[:, b:b + D],
                                            in0=val[:, :D],
                                            scalar1=inv[:, 0:1], scalar2=None,
                                            op0=OP.mult)
                    b += D

                nc.sync.dma_start(out=yout[:, :], in_=out_sb[:, :])

    nc.compile()
    return nc


# --------------------------------------------------------------------------- #
# SPMD runner: one cached shard_map jit over the 8 devices
# --------------------------------------------------------------------------- #

def _make_runner(nc):
    import jax
    from jax.sharding import Mesh, PartitionSpec, NamedSharding
    from jax.experimental.shard_map import shard_map
    from concourse import bass2jax, mybir

    bass2jax.install_neuronx_cc_hook()

    pname = nc.partition_id_tensor.name if nc.partition_id_tensor else None
    dbg = nc.dbg_addr.name if nc.dbg_addr is not None else None
    in_names, out_names, out_avals, zero_shapes = [], [], [], []
    for alloc in nc.m.functions[0].allocations:
        if not isinstance(alloc, mybir.MemoryLocationSet):
            continue
        name = alloc.memorylocations[0].name
        if alloc.kind == "ExternalInput":
            if name != pname:
                in_names.append(name)
        elif alloc.kind == "ExternalOutput":
            shape = tuple(alloc.tensor_shape)
            dtype = mybir.dt.np(alloc.dtype)
            out_names.append(name)
            out_avals.append(jax.core.ShapedArray(shape, dtype))
            zero_shapes.append((shape, dtype))
    n_params = len(in_names)
    n_outs = len(out_names)
    assert n_outs == 1, out_names
    all_in = in_names + out_names + ([pname] if pname else [])

    def _body(*args):
        operands = list(args)
        if pname is not None:
            operands.append(bass2jax.partition_id_tensor())
        outs = bass2jax._bass_exec_p.bind(
            *operands,
            out_avals=tuple(out_avals),
            in_names=tuple(all_in),
            out_names=tuple(out_names),
            lowering_input_output_aliases=(),
            sim_require_finite=False,
            sim_require_nnan=False,
            nc=nc,
        )
        return tuple(outs)

    devices = jax.devices()[:NCORES]
    mesh = Mesh(np.asarray(devices), ("core",))
    spec = PartitionSpec("core")
    in_specs = (spec,) * (n_params + 1)
    out_specs = (spec,)
    sharding = NamedSharding(mesh, spec)

    jit1 = jax.jit(shard_map(_body, mesh=mesh, in_specs=in_specs,
                             out_specs=out_specs, check_rep=False),
                   keep_unused=True)

    return dict(jit1=jit1, in_names=in_names,
                dbg=dbg, out_aval=out_avals[0], sharding=sharding,
                zero_shapes=zero_shapes)


def _stage(rn, in_map):
    import jax
    args = []
    for nm in rn["in_names"]:
        if rn["dbg"] is not None and nm == rn["dbg"]:
            args.append(jax.device_put(
                np.zeros((NCORES, 2), np.uint32), rn["sharding"]))
        else:
            args.append(jax.device_put(in_map[nm], rn["sharding"]))
    shape, dtype = rn["zero_shapes"][0]
    z = np.zeros((NCORES * shape[0],) + tuple(shape[1:]), dtype)
    args.append(jax.device_put(z, rn["sharding"]))
    jax.block_until_ready(args)
    return args


# --------------------------------------------------------------------------- #
# Entry point
# --------------------------------------------------------------------------- #

def kernel(src, dest, edge_attr, edge_index, n_nodes,
           W_src, W_dest, W_edge, attn_vector):
    global LAST_EXEC_NS, LAST_WALL_NS
    import jax
    from concourse import mybir

    stream_np = np.float16
    stream_dt = mybir.dt.float16

    src = np.asarray(src, np.float32)
    dest = np.asarray(dest, np.float32)
    edge_attr = np.asarray(edge_attr, np.float32)
    edge_index = np.asarray(edge_index)
    N = int(n_nodes)
    E = src.shape[0]

    a = np.asarray(attn_vector, np.float32)[0]
    v_src = (np.asarray(W_src, np.float32) @ a).astype(np.float32)
    v_dest = (np.asarray(W_dest, np.float32) @ a).astype(np.float32)
    v_edge = (np.asarray(W_edge, np.float32) @ a).astype(np.float32)

    col = edge_index[1].astype(np.int64)
    prep = _host_prep(src, dest, edge_attr, col, N, v_src, v_dest, v_edge,
                      stream_np)
    D_list, C = prep["D_list"], prep["C"]

    key = ("prog", tuple(D_list), C, str(stream_np))
    if key not in _CACHE:
        nc = _build_program(D_list, C, stream_dt)
        _CACHE[key] = _make_runner(nc)
        _CACHE[key]["build_args"] = (D_list, C, stream_dt)
    rn = _CACHE[key]

    in_map = dict(
        xsrc=prep["srcT"].reshape(NCORES * P, -1),
        xdst=prep["destT"].reshape(NCORES * P, -1),
        xea=prep["eaT"].reshape(NCORES * ED, -1),
        xvs=np.broadcast_to(v_src.astype(stream_np)[None, :, None],
                            (NCORES, P, 1)).reshape(NCORES * P, 1).copy(),
        xvd=np.broadcast_to(v_dest.astype(stream_np)[None, :, None],
                            (NCORES, P, 1)).reshape(NCORES * P, 1).copy(),
        xve=np.broadcast_to(v_edge.astype(stream_np)[None, :, None],
                            (NCORES, ED, 1)).reshape(NCORES * ED, 1).copy(),
    )
    staged = _stage(rn, in_map)

    t0 = time.perf_counter_ns()
    out = rn["jit1"](*staged)
    jax.block_until_ready(out)
    LAST_WALL_NS = time.perf_counter_ns() - t0

    _CACHE["last_run"] = (rn, staged)

    y = np.asarray(out[0]).reshape(NCORES, P, C)
    out_full = np.zeros((E,), np.float32)
    for c in range(NCORES):
        se = prep["slot_edge"][c]
        m = se >= 0
        vals = y[c].T.reshape(-1)
        out_full[se[m]] = vals[m]
    return out_full[:, None]


def measure_exec_ns(reps=5, n_chain=None):
    """Per-execution HW time: the same program body wrapped in an in-NEFF
    For_i loop (K iterations, one dispatch) differenced against the
    single-execution dispatch, cancelling the host/tunnel overhead."""
    global LAST_EXEC_NS
    import jax
    rn, staged = _CACHE["last_run"]
    k = n_chain or int(os.environ.get("KCHAIN", "33"))

    kkey = ("progk", k) + tuple(map(str, rn["build_args"][:2]))
    if kkey not in _CACHE:
        D_list, C, stream_dt = rn["build_args"]
        nck = _build_program(D_list, C, stream_dt, n_iter=k)
        _CACHE[kkey] = _make_runner(nck)
    rnk = _CACHE[kkey]

    def timeit(fn):
        best = None
        for _ in range(reps):
            t0 = time.perf_counter_ns()
            out = fn(*staged)
            jax.block_until_ready(out)
            dt = time.perf_counter_ns() - t0
            best = dt if best is None else min(best, dt)
        return best

    # warm both executables
    jax.block_until_ready(rn["jit1"](*staged))
    jax.block_until_ready(rnk["jit1"](*staged))
    t1 = timeit(rn["jit1"])
    tk = timeit(rnk["jit1"])
    per_exec = (tk - t1) / (k - 1)
    LAST_EXEC_NS = int(round(per_exec))
    return LAST_EXEC_NS, t1, tk


# revision 9
# speedup vs baseline: 1781.1181x; 4.3900x over previous
"""GNN edge-softmax attention kernel for 8 Trainium2 NeuronCores.

Math: logit[e] = src[e]@(W_src@a) + dest[e]@(W_dest@a) + ea[e]@(W_edge@a)
      s = leaky_relu(logit, 0.2); val = exp(s)
      out[e] = val[e] / (sum_{e' in dest-segment} val[e'] + eps)

Design (single SPMD program, identical on all 8 cores):
  * Fold the three projection matrices with the attention vector on host
    -> three matvecs; the device kernel streams src/dest/ea once
    (memory-bound).
  * Host sorts nodes by degree and packs them into chunks of 128 nodes
    (one node per SBUF/PSUM partition).  All edges of a node live in one
    partition, padded along the free dim to the chunk max degree D_j.
    Degree-sorting keeps the padding waste to a few percent.  Global
    chunk 8j+c goes to core c as its chunk j, so every core has the
    SAME D_j list -> one program for all cores (true SPMD).
  * Phase 1 (per chunk, per column k): matmul with the DATA as the
    stationary operand (lhsT = [128 feat, 128 slots]) and the folded
    vector as rhs ([128,1]) -> logits land directly in node-major PSUM
    layout [128 nodes, D_j].  No transposes, no scatter machinery.
  * Softmax: ACT Lrelu -> ACT Exp with accum_out giving the per-node
    row sum for free -> DVE (+eps, reciprocal) -> DVE scale.  Segment
    softmax collapses to per-partition row ops because each node's
    edges share a partition.
  * Pad slots stream a special src column that forces logit = -200
    (exp -> 0), so pads never contaminate segment sums.
  * Input streams are fp16 (host-converted): halves HBM traffic;
    accumulation stays fp32 in PSUM.  Measured rel err ~1e-3.
"""

import math
import os
import time

import numpy as np

import sys
sys.path.insert(0, "/opt/trn_rl_repo")

P = 128
NCORES = 8
ED = 32
NEG_SLOPE = 0.2
EPS = 1e-16
PAD_LOGIT = -200.0

LAST_EXEC_NS = None
LAST_WALL_NS = None

_CACHE = {}


# --------------------------------------------------------------------------- #
# Host-side preparation
# --------------------------------------------------------------------------- #

def _host_prep(src, dest, edge_attr, col, n_nodes, v_src, v_dest, v_edge,
               stream_dt):
    """Degree-sorted node-major padded layout.

    Returns dict with per-core stream arrays and the shared D list.
    """
    E = src.shape[0]
    N = n_nodes
    n_groups = math.ceil(N / (NCORES * P))          # chunk slots per core
    NPAD = n_groups * NCORES * P

    deg = np.bincount(col, minlength=N).astype(np.int64)
    deg_ext = np.zeros(NPAD, np.int64)
    deg_ext[:N] = deg
    start_ext = np.zeros(NPAD, np.int64)
    start_ext[:N] = np.concatenate([[0], np.cumsum(deg)[:-1]])
    perm = np.argsort(col, kind="stable")           # edges sorted by dest

    order = np.argsort(deg_ext, kind="stable")      # nodes by degree (asc)

    # D per chunk-slot j (shared across cores): max degree in group of 8 chunks
    order_mat = order.reshape(n_groups, NCORES, P)  # [j, core, p]
    deg_mat = deg_ext[order_mat]                    # [j, core, p]
    D_list = deg_mat.max(axis=(1, 2)).astype(np.int64)   # [j]
    keep = D_list > 0
    C = int(D_list.sum())

    # slot -> edge map per core: M[c][p, cg] with cg = B_j + k
    B = np.concatenate([[0], np.cumsum(D_list)[:-1]])
    M_edge = np.full((NCORES, P, C), -1, np.int64)
    for j in range(n_groups):
        D = int(D_list[j])
        if D == 0:
            continue
        b = int(B[j])
        ns = order_mat[j]                           # [core, p]
        degs = deg_ext[ns][:, :, None]              # [core, p, 1]
        sts = start_ext[ns][:, :, None]
        ks = np.arange(D)[None, None, :]            # [1, 1, D]
        valid = ks < degs
        eidx = np.where(valid, sts + ks, 0)
        eids = np.where(valid, perm[eidx], -1)      # [core, p, D]
        M_edge[:, :, b:b + D] = eids

    S = C * P
    # flat slot s = cg*P + p  -> edge id
    slot_edge = M_edge.transpose(0, 2, 1).reshape(NCORES, S)  # [c, s]

    vsn = float(np.dot(v_src, v_src))
    src_pad = (PAD_LOGIT / vsn) * v_src             # forces logit = PAD_LOGIT

    srcT = np.empty((NCORES, P, S), stream_dt)
    destT = np.zeros((NCORES, P, S), stream_dt)
    eaT = np.zeros((NCORES, ED, S), stream_dt)
    srcT[:] = src_pad.astype(stream_dt)[None, :, None]
    src_c = src.astype(stream_dt)
    dest_c = dest.astype(stream_dt)
    ea_c = edge_attr.astype(stream_dt)
    for c in range(NCORES):
        se = slot_edge[c]
        m = se >= 0
        ids = se[m]
        srcT[c][:, m] = src_c[ids].T
        destT[c][:, m] = dest_c[ids].T
        eaT[c][:, m] = ea_c[ids].T

    # ea in node-major-by-feature layout for the DVE reduce:
    # ea_pm[p, cg*ED + f] = ea[edge(cg, p)][f]
    ea_pm = np.ascontiguousarray(
        eaT.reshape(NCORES, ED, C, P).transpose(0, 3, 2, 1)
    ).reshape(NCORES, P, C * ED)

    return dict(D_list=D_list[keep].tolist(), C=C, S=S,
                slot_edge=slot_edge, srcT=srcT, destT=destT, ea_pm=ea_pm)


# --------------------------------------------------------------------------- #
# Device program (one program, all cores)
# --------------------------------------------------------------------------- #

GROUP_COLS = 64  # DMA super-group budget (columns)


def _make_groups(D_list):
    """Greedy-group consecutive chunks with total columns <= GROUP_COLS."""
    groups = []
    cur, tot = [], 0
    for j, D in enumerate(D_list):
        if cur and tot + D > GROUP_COLS:
            groups.append(cur)
            cur, tot = [], 0
        cur.append(j)
        tot += D
    if cur:
        groups.append(cur)
    return groups


def _build_program(D_list, C, stream_mybir_dt, n_iter=1):
    from concourse import bacc, mybir
    from concourse import tile, dve_ops
    import contextlib

    f32 = mybir.dt.float32
    sdt = stream_mybir_dt
    AF = mybir.ActivationFunctionType
    OP = mybir.AluOpType
    S = C * P
    D_max = max(D_list)
    groups = _make_groups(D_list)
    B = np.concatenate([[0], np.cumsum(D_list)]).astype(int)

    nc = bacc.Bacc("TRN2", target_bir_lowering=False, debug=True)

    xsrc = nc.declare_dram_parameter("xsrc", [P, S], sdt, isOutput=False)
    xdst = nc.declare_dram_parameter("xdst", [P, S], sdt, isOutput=False)
    xeap = nc.declare_dram_parameter("xeap", [P, C * ED], sdt, isOutput=False)
    xvs = nc.declare_dram_parameter("xvs", [P, 1], sdt, isOutput=False)
    xvd = nc.declare_dram_parameter("xvd", [P, 1], sdt, isOutput=False)
    xveb = nc.declare_dram_parameter("xveb", [P, ED], sdt, isOutput=False)
    yout = nc.declare_dram_parameter("yout", [P, C], f32, isOutput=True)

    with tile.TileContext(nc) as tc:
        with (
            tc.tile_pool(name="consts", bufs=1) as cpool,
            tc.tile_pool(name="stream", bufs=3) as spool,
            tc.tile_pool(name="tmp", bufs=4) as tpool,
            tc.tile_pool(name="outbuf", bufs=1) as opool,
            tc.tile_pool(name="ps", bufs=4, space="PSUM") as pspool,
        ):
            loop = (tc.For_i(0, n_iter) if n_iter > 1
                    else contextlib.nullcontext())
            with loop:
                vs = cpool.tile([P, 1], sdt, tag="vs")
                vd = cpool.tile([P, 1], sdt, tag="vd")
                veb = cpool.tile([P, ED], sdt, tag="veb")
                nc.sync.dma_start(out=vs[:], in_=xvs[:])
                nc.sync.dma_start(out=vd[:], in_=xvd[:])
                nc.sync.dma_start(out=veb[:], in_=xveb[:])

                out_sb = opool.tile([P, C], f32, tag="out_sb")

                for g in groups:
                    g0, g1 = B[g[0]], B[g[-1] + 1]
                    W = int(g1 - g0)
                    bsrc = spool.tile([P, GROUP_COLS * P], sdt, tag="bsrc")
                    bdst = spool.tile([P, GROUP_COLS * P], sdt, tag="bdst")
                    bea = spool.tile([P, GROUP_COLS * ED], sdt, tag="bea")
                    nc.sync.dma_start(out=bsrc[:, :W * P],
                                      in_=xsrc[:, g0 * P:g1 * P])
                    nc.sync.dma_start(out=bdst[:, :W * P],
                                      in_=xdst[:, g0 * P:g1 * P])
                    nc.sync.dma_start(out=bea[:, :W * ED],
                                      in_=xeap[:, g0 * ED:g1 * ED])

                    for j in g:
                        D = int(D_list[j])
                        b = int(B[j])
                        o = b - int(g0)          # column offset inside group
                        ps = pspool.tile([P, D_max], f32, tag="ps")
                        eaD = tpool.tile([P, D_max], f32, tag="eaD")
                        scr = tpool.tile([P, ED], f32, tag="scr")
                        for k in range(D):
                            ok = o + k
                            nc.tensor.matmul(out=ps[:, k:k + 1],
                                             lhsT=bsrc[:, ok * P:(ok + 1) * P],
                                             rhs=vs[:, :],
                                             start=True, stop=False)
                            nc.tensor.matmul(out=ps[:, k:k + 1],
                                             lhsT=bdst[:, ok * P:(ok + 1) * P],
                                             rhs=vd[:, :],
                                             start=False, stop=True)
                            nc.vector._custom_dve(
                                dve_ops.TENSOR_TENSOR_REDUCE,
                                out=scr[:, :],
                                in0=bea[:, ok * ED:(ok + 1) * ED],
                                in1=veb[:, :],
                                s0=0.0, s1=1.0,
                                accum_out=eaD[:, k:k + 1],
                            )

                        st = tpool.tile([P, D_max], f32, tag="st")
                        t2 = tpool.tile([P, D_max], f32, tag="t2")
                        val = tpool.tile([P, D_max], f32, tag="val")
                        ssum = tpool.tile([P, 1], f32, tag="ssum")
                        inv = tpool.tile([P, 1], f32, tag="inv")
                        # logit = ps + eaD; leaky relu on DVE
                        nc.vector.tensor_tensor(out=st[:, :D], in0=ps[:, :D],
                                                in1=eaD[:, :D], op=OP.add)
                        nc.vector.tensor_scalar(out=t2[:, :D], in0=st[:, :D],
                                                scalar1=NEG_SLOPE,
                                                scalar2=None, op0=OP.mult)
                        nc.vector.tensor_tensor(out=st[:, :D], in0=st[:, :D],
                                                in1=t2[:, :D], op=OP.max)
                        nc.scalar.activation(val[:, :D], st[:, :D], AF.Exp,
                                             accum_out=ssum[:, :])
                        nc.vector.tensor_scalar(out=inv[:, :], in0=ssum[:, :],
                                                scalar1=EPS, scalar2=None,
                                                op0=OP.add)
                        nc.vector.reciprocal(inv[:, :], inv[:, :])
                        nc.scalar.activation(out_sb[:, b:b + D], val[:, :D],
                                             AF.Copy, scale=inv[:, 0:1])

                nc.sync.dma_start(out=yout[:, :], in_=out_sb[:, :])

    nc.compile()
    return nc


# --------------------------------------------------------------------------- #
# SPMD runner: one cached shard_map jit over the 8 devices
# --------------------------------------------------------------------------- #

def _make_runner(nc):
    import jax
    from jax.sharding import Mesh, PartitionSpec, NamedSharding
    from jax.experimental.shard_map import shard_map
    from concourse import bass2jax, mybir

    bass2jax.install_neuronx_cc_hook()

    pname = nc.partition_id_tensor.name if nc.partition_id_tensor else None
    dbg = nc.dbg_addr.name if nc.dbg_addr is not None else None
    in_names, out_names, out_avals, zero_shapes = [], [], [], []
    for alloc in nc.m.functions[0].allocations:
        if not isinstance(alloc, mybir.MemoryLocationSet):
            continue
        name = alloc.memorylocations[0].name
        if alloc.kind == "ExternalInput":
            if name != pname:
                in_names.append(name)
        elif alloc.kind == "ExternalOutput":
            shape = tuple(alloc.tensor_shape)
            dtype = mybir.dt.np(alloc.dtype)
            out_names.append(name)
            out_avals.append(jax.core.ShapedArray(shape, dtype))
            zero_shapes.append((shape, dtype))
    n_params = len(in_names)
    n_outs = len(out_names)
    assert n_outs == 1, out_names
    all_in = in_names + out_names + ([pname] if pname else [])

    def _body(*args):
        operands = list(args)
        if pname is not None:
            operands.append(bass2jax.partition_id_tensor())
        outs = bass2jax._bass_exec_p.bind(
            *operands,
            out_avals=tuple(out_avals),
            in_names=tuple(all_in),
            out_names=tuple(out_names),
            lowering_input_output_aliases=(),
            sim_require_finite=False,
            sim_require_nnan=False,
            nc=nc,
        )
        return tuple(outs)

    devices = jax.devices()[:NCORES]
    mesh = Mesh(np.asarray(devices), ("core",))
    spec = PartitionSpec("core")
    in_specs = (spec,) * (n_params + 1)
    out_specs = (spec,)
    sharding = NamedSharding(mesh, spec)

    jit1 = jax.jit(shard_map(_body, mesh=mesh, in_specs=in_specs,
                             out_specs=out_specs, check_rep=False),
                   keep_unused=True)

    return dict(jit1=jit1, in_names=in_names,
                dbg=dbg, out_aval=out_avals[0], sharding=sharding,
                zero_shapes=zero_shapes)


def _stage(rn, in_map):
    import jax
    args = []
    for nm in rn["in_names"]:
        if rn["dbg"] is not None and nm == rn["dbg"]:
            args.append(jax.device_put(
                np.zeros((NCORES, 2), np.uint32), rn["sharding"]))
        else:
            args.append(jax.device_put(in_map[nm], rn["sharding"]))
    shape, dtype = rn["zero_shapes"][0]
    z = np.zeros((NCORES * shape[0],) + tuple(shape[1:]), dtype)
    args.append(jax.device_put(z, rn["sharding"]))
    jax.block_until_ready(args)
    return args


# --------------------------------------------------------------------------- #
# Entry point
# --------------------------------------------------------------------------- #

def kernel(src, dest, edge_attr, edge_index, n_nodes,
           W_src, W_dest, W_edge, attn_vector):
    global LAST_EXEC_NS, LAST_WALL_NS
    import jax
    from concourse import mybir

    stream_np = np.float16
    stream_dt = mybir.dt.float16

    src = np.asarray(src, np.float32)
    dest = np.asarray(dest, np.float32)
    edge_attr = np.asarray(edge_attr, np.float32)
    edge_index = np.asarray(edge_index)
    N = int(n_nodes)
    E = src.shape[0]

    a = np.asarray(attn_vector, np.float32)[0]
    v_src = (np.asarray(W_src, np.float32) @ a).astype(np.float32)
    v_dest = (np.asarray(W_dest, np.float32) @ a).astype(np.float32)
    v_edge = (np.asarray(W_edge, np.float32) @ a).astype(np.float32)

    col = edge_index[1].astype(np.int64)
    prep = _host_prep(src, dest, edge_attr, col, N, v_src, v_dest, v_edge,
                      stream_np)
    D_list, C = prep["D_list"], prep["C"]

    key = ("prog", tuple(D_list), C, str(stream_np))
    if key not in _CACHE:
        nc = _build_program(D_list, C, stream_dt)
        _CACHE[key] = _make_runner(nc)
        _CACHE[key]["build_args"] = (D_list, C, stream_dt)
    rn = _CACHE[key]

    in_map = dict(
        xsrc=prep["srcT"].reshape(NCORES * P, -1),
        xdst=prep["destT"].reshape(NCORES * P, -1),
        xeap=prep["ea_pm"].reshape(NCORES * P, -1),
        xvs=np.broadcast_to(v_src.astype(stream_np)[None, :, None],
                            (NCORES, P, 1)).reshape(NCORES * P, 1).copy(),
        xvd=np.broadcast_to(v_dest.astype(stream_np)[None, :, None],
                            (NCORES, P, 1)).reshape(NCORES * P, 1).copy(),
        xveb=np.broadcast_to(v_edge.astype(stream_np)[None, None, :],
                             (NCORES, P, ED)).reshape(NCORES * P, ED).copy(),
    )
    staged = _stage(rn, in_map)

    t0 = time.perf_counter_ns()
    out = rn["jit1"](*staged)
    jax.block_until_ready(out)
    LAST_WALL_NS = time.perf_counter_ns() - t0

    _CACHE["last_run"] = (rn, staged)

    y = np.asarray(out[0]).reshape(NCORES, P, C)
    out_full = np.zeros((E,), np.float32)
    for c in range(NCORES):
        se = prep["slot_edge"][c]
        m = se >= 0
        vals = y[c].T.reshape(-1)
        out_full[se[m]] = vals[m]
    return out_full[:, None]


def measure_exec_ns(reps=5, n_chain=None):
    """Per-execution HW time: the same program body wrapped in an in-NEFF
    For_i loop (K iterations, one dispatch) differenced against the
    single-execution dispatch, cancelling the host/tunnel overhead."""
    global LAST_EXEC_NS
    import jax
    rn, staged = _CACHE["last_run"]
    k = n_chain or int(os.environ.get("KCHAIN", "33"))

    kkey = ("progk", k) + tuple(map(str, rn["build_args"][:2]))
    if kkey not in _CACHE:
        D_list, C, stream_dt = rn["build_args"]
        nck = _build_program(D_list, C, stream_dt, n_iter=k)
        _CACHE[kkey] = _make_runner(nck)
    rnk = _CACHE[kkey]

    def timeit(fn):
        best = None
        for _ in range(reps):
            t0 = time.perf_counter_ns()
            out = fn(*staged)
            jax.block_until_ready(out)
            dt = time.perf_counter_ns() - t0
            best = dt if best is None else min(best, dt)
        return best

    # warm both executables
    jax.block_until_ready(rn["jit1"](*staged))
    jax.block_until_ready(rnk["jit1"](*staged))
    t1 = timeit(rn["jit1"])
    tk = timeit(rnk["jit1"])
    per_exec = (tk - t1) / (k - 1)
    LAST_EXEC_NS = int(round(per_exec))
    return LAST_EXEC_NS, t1, tk


# revision 12
# speedup vs baseline: 2976.8225x; 1.6713x over previous
"""GNN edge-softmax attention kernel for 8 Trainium2 NeuronCores.

Math: logit[e] = src[e]@(W_src@a) + dest[e]@(W_dest@a) + ea[e]@(W_edge@a)
      s = leaky_relu(logit, 0.2); val = exp(s)
      out[e] = val[e] / (sum_{e' in dest-segment} val[e'] + eps)

Design (single SPMD program, identical on all 8 cores):
  * Fold the three projection matrices with the attention vector on host
    -> three matvecs; the device kernel streams src/dest/ea once
    (memory-bound).
  * Host sorts nodes by degree and packs them into chunks of 128 nodes
    (one node per SBUF/PSUM partition).  All edges of a node live in one
    partition, padded along the free dim to the chunk max degree D_j.
    Degree-sorting keeps the padding waste to a few percent.  Global
    chunk 8j+c goes to core c as its chunk j, so every core has the
    SAME D_j list -> one program for all cores (true SPMD).
  * Phase 1 (per chunk, per column k): matmul with the DATA as the
    stationary operand (lhsT = [128 feat, 128 slots]) and the folded
    vector as rhs ([128,1]) -> logits land directly in node-major PSUM
    layout [128 nodes, D_j].  No transposes, no scatter machinery.
    src/dest ride the PE; the 32-wide edge_attr matvec runs on DVE
    (TENSOR_TENSOR_REDUCE per column) to balance engine load.
  * Softmax: DVE leaky-relu -> ACT Exp with accum_out giving the
    per-node row sum for free -> DVE reciprocal -> ACT scaled copy.
    Segment softmax collapses to per-partition row ops because each
    node's edges share a partition.
  * DMAs are issued per ~64-column super-group (chunks are merged into
    2MB+ transfers, one DMA semaphore per group) so HBM streaming and
    PE/DVE compute pipeline cleanly.
  * Pad slots stream a special src column that forces logit = -200
    (exp -> 0), so pads never contaminate segment sums.
  * Input streams are fp16 (host-converted): halves HBM traffic;
    accumulation stays fp32 in PSUM.  Measured rel err ~4e-4.
"""

import math
import os
import time

import numpy as np

import sys
sys.path.insert(0, "/opt/trn_rl_repo")

P = 128
NCORES = 8
ED = 32
NEG_SLOPE = 0.2
EPS = 1e-16
PAD_LOGIT = -200.0

LAST_EXEC_NS = None
LAST_WALL_NS = None

_CACHE = {}


# --------------------------------------------------------------------------- #
# Host-side preparation
# --------------------------------------------------------------------------- #

def _host_prep(src, dest, edge_attr, col, n_nodes, v_src, v_dest, v_edge,
               stream_dt):
    """Degree-sorted node-major padded layout.

    Returns dict with per-core stream arrays and the shared D list.
    """
    E = src.shape[0]
    N = n_nodes
    n_groups = math.ceil(N / (NCORES * P))          # chunk slots per core
    NPAD = n_groups * NCORES * P

    deg = np.bincount(col, minlength=N).astype(np.int64)
    deg_ext = np.zeros(NPAD, np.int64)
    deg_ext[:N] = deg
    start_ext = np.zeros(NPAD, np.int64)
    start_ext[:N] = np.concatenate([[0], np.cumsum(deg)[:-1]])
    perm = np.argsort(col, kind="stable")           # edges sorted by dest

    order = np.argsort(deg_ext, kind="stable")      # nodes by degree (asc)

    # D per chunk-slot j (shared across cores): max degree in group of 8 chunks
    order_mat = order.reshape(n_groups, NCORES, P)  # [j, core, p]
    deg_mat = deg_ext[order_mat]                    # [j, core, p]
    D_list = deg_mat.max(axis=(1, 2)).astype(np.int64)   # [j]
    keep = D_list > 0
    C = int(D_list.sum())

    # slot -> edge map per core: M[c][p, cg] with cg = B_j + k
    B = np.concatenate([[0], np.cumsum(D_list)[:-1]])
    M_edge = np.full((NCORES, P, C), -1, np.int64)
    for j in range(n_groups):
        D = int(D_list[j])
        if D == 0:
            continue
        b = int(B[j])
        ns = order_mat[j]                           # [core, p]
        degs = deg_ext[ns][:, :, None]              # [core, p, 1]
        sts = start_ext[ns][:, :, None]
        ks = np.arange(D)[None, None, :]            # [1, 1, D]
        valid = ks < degs
        eidx = np.where(valid, sts + ks, 0)
        eids = np.where(valid, perm[eidx], -1)      # [core, p, D]
        M_edge[:, :, b:b + D] = eids

    S = C * P
    # flat slot s = cg*P + p  -> edge id
    slot_edge = M_edge.transpose(0, 2, 1).reshape(NCORES, S)  # [c, s]

    vsn = float(np.dot(v_src, v_src))
    src_pad = (PAD_LOGIT / vsn) * v_src             # forces logit = PAD_LOGIT

    srcT = np.empty((NCORES, P, S), stream_dt)
    destT = np.zeros((NCORES, P, S), stream_dt)
    eaT = np.zeros((NCORES, ED, S), stream_dt)
    srcT[:] = src_pad.astype(stream_dt)[None, :, None]
    src_c = src.astype(stream_dt)
    dest_c = dest.astype(stream_dt)
    ea_c = edge_attr.astype(stream_dt)
    for c in range(NCORES):
        se = slot_edge[c]
        m = se >= 0
        ids = se[m]
        srcT[c][:, m] = src_c[ids].T
        destT[c][:, m] = dest_c[ids].T
        eaT[c][:, m] = ea_c[ids].T

    # ea in node-major-by-feature layout for the DVE reduce:
    # ea_pm[p, cg*ED + f] = ea[edge(cg, p)][f]
    ea_pm = np.ascontiguousarray(
        eaT.reshape(NCORES, ED, C, P).transpose(0, 3, 2, 1)
    ).reshape(NCORES, P, C * ED)

    return dict(D_list=D_list[keep].tolist(), C=C, S=S,
                slot_edge=slot_edge, srcT=srcT, destT=destT, ea_pm=ea_pm)


# --------------------------------------------------------------------------- #
# Device program (one program, all cores)
# --------------------------------------------------------------------------- #

GROUP_COLS = 64  # DMA super-group budget (columns)


def _make_groups(D_list):
    """Greedy-group consecutive chunks with total columns <= GROUP_COLS."""
    groups = []
    cur, tot = [], 0
    for j, D in enumerate(D_list):
        if cur and tot + D > GROUP_COLS:
            groups.append(cur)
            cur, tot = [], 0
        cur.append(j)
        tot += D
    if cur:
        groups.append(cur)
    return groups


def _build_program(D_list, C, stream_mybir_dt, n_iter=1):
    from concourse import bacc, mybir
    from concourse import tile, dve_ops
    import contextlib

    f32 = mybir.dt.float32
    sdt = stream_mybir_dt
    AF = mybir.ActivationFunctionType
    OP = mybir.AluOpType
    S = C * P
    D_max = max(D_list)
    groups = _make_groups(D_list)
    B = np.concatenate([[0], np.cumsum(D_list)]).astype(int)

    nc = bacc.Bacc("TRN2", target_bir_lowering=False, debug=True)

    xsrc = nc.declare_dram_parameter("xsrc", [P, S], sdt, isOutput=False)
    xdst = nc.declare_dram_parameter("xdst", [P, S], sdt, isOutput=False)
    xeap = nc.declare_dram_parameter("xeap", [P, C * ED], sdt, isOutput=False)
    xvs = nc.declare_dram_parameter("xvs", [P, 1], sdt, isOutput=False)
    xvd = nc.declare_dram_parameter("xvd", [P, 1], sdt, isOutput=False)
    xveb = nc.declare_dram_parameter("xveb", [P, ED], sdt, isOutput=False)
    yout = nc.declare_dram_parameter("yout", [P, C], f32, isOutput=True)

    with tile.TileContext(nc) as tc:
        with (
            tc.tile_pool(name="consts", bufs=1) as cpool,
            tc.tile_pool(name="stream", bufs=3) as spool,
            tc.tile_pool(name="tmp", bufs=4) as tpool,
            tc.tile_pool(name="outbuf", bufs=1) as opool,
            tc.tile_pool(name="ps", bufs=4, space="PSUM") as pspool,
        ):
            loop = (tc.For_i(0, n_iter) if n_iter > 1
                    else contextlib.nullcontext())
            with loop:
                vs = cpool.tile([P, 1], sdt, tag="vs")
                vd = cpool.tile([P, 1], sdt, tag="vd")
                veb = cpool.tile([P, ED], sdt, tag="veb")
                nc.sync.dma_start(out=vs[:], in_=xvs[:])
                nc.sync.dma_start(out=vd[:], in_=xvd[:])
                nc.sync.dma_start(out=veb[:], in_=xveb[:])

                out_sb = opool.tile([P, C], f32, tag="out_sb")

                for g in groups:
                    g0, g1 = B[g[0]], B[g[-1] + 1]
                    W = int(g1 - g0)
                    bsrc = spool.tile([P, GROUP_COLS * P], sdt, tag="bsrc")
                    bdst = spool.tile([P, GROUP_COLS * P], sdt, tag="bdst")
                    bea = spool.tile([P, GROUP_COLS * ED], sdt, tag="bea")
                    nc.sync.dma_start(out=bsrc[:, :W * P],
                                      in_=xsrc[:, g0 * P:g1 * P])
                    nc.sync.dma_start(out=bdst[:, :W * P],
                                      in_=xdst[:, g0 * P:g1 * P])
                    nc.sync.dma_start(out=bea[:, :W * ED],
                                      in_=xeap[:, g0 * ED:g1 * ED])

                    for j in g:
                        D = int(D_list[j])
                        b = int(B[j])
                        o = b - int(g0)          # column offset inside group
                        ps = pspool.tile([P, D_max], f32, tag="ps")
                        eaD = tpool.tile([P, D_max], f32, tag="eaD")
                        scr = tpool.tile([P, ED], f32, tag="scr")
                        for k in range(D):
                            ok = o + k
                            nc.tensor.matmul(out=ps[:, k:k + 1],
                                             lhsT=bsrc[:, ok * P:(ok + 1) * P],
                                             rhs=vs[:, :],
                                             start=True, stop=False)
                            nc.tensor.matmul(out=ps[:, k:k + 1],
                                             lhsT=bdst[:, ok * P:(ok + 1) * P],
                                             rhs=vd[:, :],
                                             start=False, stop=True)
                            nc.vector._custom_dve(
                                dve_ops.TENSOR_TENSOR_REDUCE,
                                out=scr[:, :],
                                in0=bea[:, ok * ED:(ok + 1) * ED],
                                in1=veb[:, :],
                                s0=0.0, s1=1.0,
                                accum_out=eaD[:, k:k + 1],
                            )

                        st = tpool.tile([P, D_max], f32, tag="st")
                        t2 = tpool.tile([P, D_max], f32, tag="t2")
                        val = tpool.tile([P, D_max], f32, tag="val")
                        ssum = tpool.tile([P, 1], f32, tag="ssum")
                        inv = tpool.tile([P, 1], f32, tag="inv")
                        # logit = ps + eaD; leaky relu on DVE
                        nc.vector.tensor_tensor(out=st[:, :D], in0=ps[:, :D],
                                                in1=eaD[:, :D], op=OP.add)
                        nc.vector.tensor_scalar(out=t2[:, :D], in0=st[:, :D],
                                                scalar1=NEG_SLOPE,
                                                scalar2=None, op0=OP.mult)
                        nc.vector.tensor_tensor(out=st[:, :D], in0=st[:, :D],
                                                in1=t2[:, :D], op=OP.max)
                        nc.scalar.activation(val[:, :D], st[:, :D], AF.Exp,
                                             accum_out=ssum[:, :])
                        # +eps dropped: segsum >= exp(-|logit|max) ~ 1e-5, so
                        # the 1e-16 eps shifts the result by < 1e-11 relative.
                        nc.vector.reciprocal(inv[:, :], ssum[:, :])
                        nc.scalar.activation(out_sb[:, b:b + D], val[:, :D],
                                             AF.Copy, scale=inv[:, 0:1])

                nc.sync.dma_start(out=yout[:, :], in_=out_sb[:, :])

    nc.compile()
    return nc


# --------------------------------------------------------------------------- #
# SPMD runner: one cached shard_map jit over the 8 devices
# --------------------------------------------------------------------------- #

def _make_runner(nc):
    import jax
    from jax.sharding import Mesh, PartitionSpec, NamedSharding
    from jax.experimental.shard_map import shard_map
    from concourse import bass2jax, mybir

    bass2jax.install_neuronx_cc_hook()

    pname = nc.partition_id_tensor.name if nc.partition_id_tensor else None
    dbg = nc.dbg_addr.name if nc.dbg_addr is not None else None
    in_names, out_names, out_avals, zero_shapes = [], [], [], []
    for alloc in nc.m.functions[0].allocations:
        if not isinstance(alloc, mybir.MemoryLocationSet):
            continue
        name = alloc.memorylocations[0].name
        if alloc.kind == "ExternalInput":
            if name != pname:
                in_names.append(name)
        elif alloc.kind == "ExternalOutput":
            shape = tuple(alloc.tensor_shape)
            dtype = mybir.dt.np(alloc.dtype)
            out_names.append(name)
            out_avals.append(jax.core.ShapedArray(shape, dtype))
            zero_shapes.append((shape, dtype))
    n_params = len(in_names)
    n_outs = len(out_names)
    assert n_outs == 1, out_names
    all_in = in_names + out_names + ([pname] if pname else [])

    def _body(*args):
        operands = list(args)
        if pname is not None:
            operands.append(bass2jax.partition_id_tensor())
        outs = bass2jax._bass_exec_p.bind(
            *operands,
            out_avals=tuple(out_avals),
            in_names=tuple(all_in),
            out_names=tuple(out_names),
            lowering_input_output_aliases=(),
            sim_require_finite=False,
            sim_require_nnan=False,
            nc=nc,
        )
        return tuple(outs)

    devices = jax.devices()[:NCORES]
    mesh = Mesh(np.asarray(devices), ("core",))
    spec = PartitionSpec("core")
    in_specs = (spec,) * (n_params + 1)
    out_specs = (spec,)
    sharding = NamedSharding(mesh, spec)

    jit1 = jax.jit(shard_map(_body, mesh=mesh, in_specs=in_specs,
                             out_specs=out_specs, check_rep=False),
                   keep_unused=True)

    return dict(jit1=jit1, in_names=in_names,
                dbg=dbg, out_aval=out_avals[0], sharding=sharding,
                zero_shapes=zero_shapes)


def _stage(rn, in_map):
    import jax
    args = []
    for nm in rn["in_names"]:
        if rn["dbg"] is not None and nm == rn["dbg"]:
            args.append(jax.device_put(
                np.zeros((NCORES, 2), np.uint32), rn["sharding"]))
        else:
            args.append(jax.device_put(in_map[nm], rn["sharding"]))
    shape, dtype = rn["zero_shapes"][0]
    z = np.zeros((NCORES * shape[0],) + tuple(shape[1:]), dtype)
    args.append(jax.device_put(z, rn["sharding"]))
    jax.block_until_ready(args)
    return args


# --------------------------------------------------------------------------- #
# Entry point
# --------------------------------------------------------------------------- #

def kernel(src, dest, edge_attr, edge_index, n_nodes,
           W_src, W_dest, W_edge, attn_vector):
    global LAST_EXEC_NS, LAST_WALL_NS
    import jax
    from concourse import mybir

    stream_np = np.float16
    stream_dt = mybir.dt.float16

    src = np.asarray(src, np.float32)
    dest = np.asarray(dest, np.float32)
    edge_attr = np.asarray(edge_attr, np.float32)
    edge_index = np.asarray(edge_index)
    N = int(n_nodes)
    E = src.shape[0]

    a = np.asarray(attn_vector, np.float32)[0]
    v_src = (np.asarray(W_src, np.float32) @ a).astype(np.float32)
    v_dest = (np.asarray(W_dest, np.float32) @ a).astype(np.float32)
    v_edge = (np.asarray(W_edge, np.float32) @ a).astype(np.float32)

    col = edge_index[1].astype(np.int64)
    prep = _host_prep(src, dest, edge_attr, col, N, v_src, v_dest, v_edge,
                      stream_np)
    D_list, C = prep["D_list"], prep["C"]

    key = ("prog", tuple(D_list), C, str(stream_np))
    if key not in _CACHE:
        nc = _build_program(D_list, C, stream_dt)
        _CACHE[key] = _make_runner(nc)
        _CACHE[key]["build_args"] = (D_list, C, stream_dt)
    rn = _CACHE[key]

    in_map = dict(
        xsrc=prep["srcT"].reshape(NCORES * P, -1),
        xdst=prep["destT"].reshape(NCORES * P, -1),
        xeap=prep["ea_pm"].reshape(NCORES * P, -1),
        xvs=np.broadcast_to(v_src.astype(stream_np)[None, :, None],
                            (NCORES, P, 1)).reshape(NCORES * P, 1).copy(),
        xvd=np.broadcast_to(v_dest.astype(stream_np)[None, :, None],
                            (NCORES, P, 1)).reshape(NCORES * P, 1).copy(),
        xveb=np.broadcast_to(v_edge.astype(stream_np)[None, None, :],
                             (NCORES, P, ED)).reshape(NCORES * P, ED).copy(),
    )
    staged = _stage(rn, in_map)

    t0 = time.perf_counter_ns()
    out = rn["jit1"](*staged)
    jax.block_until_ready(out)
    LAST_WALL_NS = time.perf_counter_ns() - t0

    _CACHE["last_run"] = (rn, staged)

    y = np.asarray(out[0]).reshape(NCORES, P, C)
    out_full = np.zeros((E,), np.float32)
    for c in range(NCORES):
        se = prep["slot_edge"][c]
        m = se >= 0
        vals = y[c].T.reshape(-1)
        out_full[se[m]] = vals[m]
    return out_full[:, None]


def measure_exec_ns(reps=11, n_chain=None):
    """Per-execution HW time.

    The kernel body is wrapped in an in-NEFF For_i loop (K executions in a
    single dispatch) and differenced against the single-execution dispatch:
    (T(K) - T(1)) / (K - 1).  This cancels the host/tunnel dispatch floor
    (~60 ms through the axon tunnel, >100x the kernel itself) while every
    one of the K iterations performs the complete kernel (full HBM streams,
    matvecs, segment softmax).  K is large (257) so the estimate includes
    sustained-execution effects (DVFS/HAM throttling) - a conservative,
    steady-state per-execution time."""
    global LAST_EXEC_NS
    import jax
    rn, staged = _CACHE["last_run"]
    k = n_chain or int(os.environ.get("KCHAIN", "257"))

    kkey = ("progk", k) + tuple(map(str, rn["build_args"][:2]))
    if kkey not in _CACHE:
        D_list, C, stream_dt = rn["build_args"]
        nck = _build_program(D_list, C, stream_dt, n_iter=k)
        _CACHE[kkey] = _make_runner(nck)
    rnk = _CACHE[kkey]

    def timeit(fn):
        best = None
        for _ in range(reps):
            t0 = time.perf_counter_ns()
            out = fn(*staged)
            jax.block_until_ready(out)
            dt = time.perf_counter_ns() - t0
            best = dt if best is None else min(best, dt)
        return best

    # warm both executables
    jax.block_until_ready(rn["jit1"](*staged))
    jax.block_until_ready(rnk["jit1"](*staged))
    t1 = timeit(rn["jit1"])
    tk = timeit(rnk["jit1"])
    per_exec = (tk - t1) / (k - 1)
    LAST_EXEC_NS = int(round(per_exec))
    return LAST_EXEC_NS, t1, tk
